# revision 39
# baseline (speedup 1.0000x reference)
"""MinkowskiInstanceNorm (segment-reduce instance norm) on 8 Trainium2 cores.

Strategy: seg_ids are sorted, so each segment is a contiguous run of rows.
With num_segments == n_cores == 8, core j owns segment j outright — no
cross-core communication; the host slices rows per segment and stitches
outputs back.

All device traffic is bf16 (the output error budget is ~1e-2 rel; bf16
rounding is ~4e-3 worst-case), which halves HBM bytes vs f32 and lets the
whole input stay resident in SBUF, so pass 2 re-reads nothing:

  - host packs each core's slab CHANNELS-ON-PARTITIONS, partition-major
    bf16 x[128, T*2048] (partition rb*32+c, free index t*2048+j for row
    t*8192 + rb*2048 + j), so any free-dim range is a valid DMA chunk.
  - pass 1 loads in 4-tile (2 MiB) chunks for DMA efficiency, with the
    last few tiles as small chunks so the stats barrier clears early.
    Per chunk: DVE tensor_scalar(copy)+accum_out -> sum(x) (4x bf16);
    sum(x^2) split ~69/31 between ACT Square+accum_out and DVE
    tensor_tensor_reduce(x*x)+accum so both engines track the DMA rate.
  - cross-partition fold (4 row-blocks per channel) is one tiny matmul
    against a replicated 0/1 selector whose output lands per-channel
    values on ALL 128 partitions — no broadcast step; stats chain reads
    the PSUM result directly.
  - pass 2: in-place DVE tensor_scalar (x = x*A[p] - B'[p], 4x bf16) on
    ramped chunks (1,2,4,8.. tiles) stored straight out of the slab,
    alternating the two HW-DGE rings.
"""

from contextlib import ExitStack

import ml_dtypes
import numpy as np

C = 32  # channels
P = 128  # SBUF partitions
RB = P // C  # row blocks per tile (4)
FD = 2048  # rows per partition per tile (free dim)
ROWS = RB * FD  # rows per tile (8192)
NCORES = 8
EPS = 1e-8
BULK = 2  # tiles per bulk load chunk
TAIL = 2  # trailing tiles loaded singly (plus the split last tile)

BF16 = ml_dtypes.bfloat16

_PROGRAMS = {}
LAST_RESULTS = None  # BassKernelResults of the last SPMD launch (for test harness)


def _load_chunks(T):
    """Pass-1 load chunks as (start, width) in free-dim units. Bulk 4-tile
    chunks taper down (2,2,1,1) toward the end, and the last tile is split
    7/8 + 1/8, so the final stats ops are small and clear right behind the
    last bytes instead of a late 2 MiB chunk serializing 5 us of engine
    work behind it."""
    taper = []  # tile widths, built back-to-front
    rem = T - 1
    for wtiles in [1] * 6:
        if rem <= 0:
            break
        w = min(wtiles, rem)
        taper.append(w)
        rem -= w
    while rem > 0:
        w = min(BULK, rem)
        taper.append(w)
        rem -= w
    chunks = []
    a = 0
    for wtiles in reversed(taper):
        chunks.append((a, wtiles * FD))
        a += wtiles * FD
    # last tile split so its stats clear right behind the final bytes
    chunks.append((a, T * FD - a - FD // 8))
    chunks.append((T * FD - FD // 8, FD // 8))
    return chunks


def _store_chunks(T):
    """Pass-2 store chunks (in tiles): ramp up so the first store issues
    as soon as the affine coefficients land."""
    chunks = []
    a, w = 0, 1
    while a < T:
        w = min(w, T - a)
        chunks.append((a * FD, w * FD))
        a += w
        w = min(8, w * 2)
    return chunks


def _emit(nc, tc, ctx, x_d, invn_d, w_d, b_d, sel_d, selt_d, selb_d, o_d, T):
    from concourse import mybir

    dt = mybir.dt
    AX = mybir.AxisListType
    OP = mybir.AluOpType
    AF = mybir.ActivationFunctionType

    xv = x_d.ap()  # [P, T*FD] bf16
    ov = o_d.ap()

    const = ctx.enter_context(tc.tile_pool(name="const", bufs=1))
    scr = ctx.enter_context(tc.tile_pool(name="scr", bufs=1))
    psum = ctx.enter_context(tc.tile_pool(name="psum", bufs=1, space="PSUM"))

    # whole input stays resident across both passes
    slab = const.tile([P, T * FD], dt.bfloat16)

    # consts ride the scalar (ACT) DGE ring: the sync ring stays a pure
    # back-to-back stream of slab loads from instruction 0
    invn = const.tile([P, 1], dt.float32)
    nc.scalar.dma_start(out=invn[:], in_=invn_d.ap())
    wt = const.tile([P, 1], dt.float32)
    nc.scalar.dma_start(out=wt[:], in_=w_d.ap())
    bt = const.tile([P, 1], dt.float32)
    nc.scalar.dma_start(out=bt[:], in_=b_d.ap())
    # channel selector sel[p, c] = (p % 32 == c): f32 [P,32] folds the f32
    # sumsq partials; bf16 [P,32] is the stationary operand for the PE's
    # running sum(x) (32 columns -> cheap per-matmul weight load); f32
    # [32,P] broadcasts the final [32,2] affine back to all partitions
    sel = const.tile([P, C], dt.float32)
    nc.scalar.dma_start(out=sel[:], in_=sel_d.ap())
    selt = const.tile([C, P], dt.float32)
    nc.scalar.dma_start(out=selt[:], in_=selt_d.ap())
    selb = const.tile([P, C], dt.bfloat16)
    nc.scalar.dma_start(out=selb[:], in_=selb_d.ap())

    loads = _load_chunks(T)
    nL = len(loads)
    qcola = const.tile([P, nL], dt.float32)  # per-chunk sum(x^2), ACT part
    qcold = const.tile([P, nL], dt.float32)  # per-chunk sum(x^2), DVE part

    dscr = scr.tile([P, BULK * FD], dt.bfloat16)  # DVE squares
    dscr2 = scr.tile([P, BULK * FD], dt.bfloat16)  # DVE scratch (unread)
    ascr = scr.tile([P, BULK * FD], dt.bfloat16)  # ACT scratch (unread)

    epsv = const.tile([P, 1], dt.float32)
    nc.vector.memset(epsv[:], EPS)
    std = const.tile([P, 1], dt.float32)
    # dummy Sqrt up front loads sqrt_and_others (which also contains
    # square) once at t=0, instead of a mid-barrier table switch later
    nc.scalar.activation(std[:], epsv[:], AF.Sqrt, bias=epsv[:])

    # PE accumulates sum(x) (and the channel fold, via the selector) into
    # one PSUM bank: psx[c, f] += sum_p sel[p, c] * x[p, 512k + f].
    # This keeps DVE's per-chunk load under ~80% of the DMA cadence so the
    # chunk-serial latency drains instead of cascading into the barrier.
    psx = psum.tile([C, 512], dt.float32)
    nmm = sum(-(-w // 512) for _, w in loads)

    ka = kd = 0
    mm = 0
    for k, (a, w) in enumerate(loads):
        xt = slab[:, a : a + w]
        nc.sync.dma_start(out=xt, in_=xv[:, a : a + w])
        # sum(x) on the otherwise-idle PE, freeing DVE for sum(x^2)
        for c in range(0, w, 512):
            sw = min(512, w - c)
            nc.tensor.matmul(
                psx[:, :sw],
                lhsT=selb[:],
                rhs=xt[:, c : c + sw],
                start=(mm == 0),
                stop=(mm == nmm - 1),
            )
            mm += 1
        # sum(x^2) split ~48/52 ACT/DVE; shares leave both engines ~35%
        # slack so chunk-serial latency drains instead of cascading
        if w > FD // 2:
            aw = round((0.476 * w) / 128) * 128
        else:
            aw = 0 if kd <= ka else w  # tiny tail chunks: one engine each
        if aw:
            nc.scalar.activation(
                ascr[:, :aw],
                xt[:, :aw],
                AF.Square,
                accum_out=qcola[:, ka : ka + 1],
            )
            ka += 1
        if aw < w:
            # DVE: square into scratch, then sum it with the accumulator
            # (tensor_tensor_reduce is broken in this runtime)
            dw = w - aw
            nc.vector.tensor_mul(dscr[:, :dw], xt[:, aw:], xt[:, aw:])
            nc.vector.tensor_scalar(
                out=dscr2[:, :dw],
                in0=dscr[:, :dw],
                scalar1=1.0,
                scalar2=0.0,
                op0=OP.mult,
                op1=OP.add,
                accum_out=qcold[:, kd : kd + 1],
            )
            kd += 1

    # sum(x): ACT folds the PE accumulator (copy + free-dim accumulator,
    # reads PSUM directly) while DVE reduces the sumsq partials in parallel
    sx = const.tile([C, 1], dt.float32)
    pscr = scr.tile([C, 512], dt.float32)  # unread
    nc.scalar.activation(pscr[:], psx[:], AF.Copy, accum_out=sx[:])

    # per-chunk sumsq partials -> [P, 1]
    qa = const.tile([P, 2], dt.float32)
    nc.vector.tensor_reduce(out=qa[:, 0:1], in_=qcola[:, :ka], axis=AX.X, op=OP.add)
    nc.vector.tensor_reduce(out=qa[:, 1:2], in_=qcold[:, :kd], axis=AX.X, op=OP.add)
    q = const.tile([P, 1], dt.float32)
    nc.vector.tensor_add(q[:], qa[:, 0:1], qa[:, 1:2])

    # fold the RB row-blocks of each channel's sumsq -> [32, 1]
    tot = psum.tile([C, 1], dt.float32)
    nc.tensor.matmul(tot[:], lhsT=sel[:], rhs=q[:], start=True, stop=True)

    # stats chain on 32 partitions
    me = const.tile([C, 2], dt.float32)
    nc.vector.tensor_scalar_mul(me[:, 0:1], sx[:], invn[:C, :])
    nc.vector.tensor_scalar_mul(me[:, 1:2], tot[:], invn[:C, :])
    mean = me[:, 0:1]
    # nvar = mean^2 - E[x^2] = -var, then std = sqrt(-nvar + eps)
    nvar = const.tile([C, 1], dt.float32)
    nc.vector.scalar_tensor_tensor(
        out=nvar[:], in0=mean, scalar=mean, in1=me[:, 1:2],
        op0=OP.mult, op1=OP.subtract,
    )
    nc.scalar.activation(
        std[:C, :], nvar[:], AF.Sqrt, bias=epsv[:C, :], scale=-1.0
    )
    istd = const.tile([C, 1], dt.float32)
    nc.vector.reciprocal(istd[:], std[:C, :])
    # ab32 = [A | B']: A = w/std, B' = mean*A - b; out = x*A - B'
    ab32 = const.tile([C, 2], dt.float32)
    nc.vector.tensor_scalar(
        out=ab32[:, 0:1], in0=wt[:C, :], scalar1=istd[:], scalar2=None,
        op0=OP.mult,
    )
    nc.vector.scalar_tensor_tensor(
        out=ab32[:, 1:2], in0=mean, scalar=ab32[:, 0:1], in1=bt[:C, :],
        op0=OP.mult, op1=OP.subtract,
    )
    # broadcast [32, 2] -> all 128 partitions (transposed-selector matmul)
    abps = psum.tile([P, 2], dt.float32)
    nc.tensor.matmul(abps[:], lhsT=selt[:], rhs=ab32[:], start=True, stop=True)
    # DVE copy: the normalize runs on DVE too, so no cross-engine sync
    ab = const.tile([P, 2], dt.float32)
    nc.vector.tensor_copy(ab[:], abps[:])

    for k, (a, w) in enumerate(_store_chunks(T)):
        xt = slab[:, a : a + w]
        # in-place normalize: x = x*A - B' (4x bf16, single-src)
        nc.vector.tensor_scalar(
            out=xt,
            in0=xt,
            scalar1=ab[:, 0:1],
            scalar2=ab[:, 1:2],
            op0=OP.mult,
            op1=OP.subtract,
        )
        # alternate the two HW-DGE rings (sync is idle during pass 2) so
        # per-store issue cost never gates the store stream
        eng = nc.scalar if k % 2 == 0 else nc.sync
        eng.dma_start(out=ov[:, a : a + w], in_=xt)


def _get_program(T):
    if T in _PROGRAMS:
        return _PROGRAMS[T]
    import concourse.tile as tile
    from concourse import bacc, mybir

    dt = mybir.dt
    nc = bacc.Bacc(
        "TRN2",
        target_bir_lowering=False,
        debug=False,
        enable_asserts=False,
        num_devices=NCORES,
    )
    x_d = nc.dram_tensor("x", [P, T * FD], dt.bfloat16, kind="ExternalInput")
    invn_d = nc.dram_tensor("invn", [P, 1], dt.float32, kind="ExternalInput")
    w_d = nc.dram_tensor("w", [P, 1], dt.float32, kind="ExternalInput")
    b_d = nc.dram_tensor("b", [P, 1], dt.float32, kind="ExternalInput")
    sel_d = nc.dram_tensor("sel", [P, C], dt.float32, kind="ExternalInput")
    selt_d = nc.dram_tensor("selt", [C, P], dt.float32, kind="ExternalInput")
    selb_d = nc.dram_tensor("selb", [P, C], dt.bfloat16, kind="ExternalInput")
    o_d = nc.dram_tensor("o", [P, T * FD], dt.bfloat16, kind="ExternalOutput")

    with tile.TileContext(nc) as tc:
        with ExitStack() as ctx:
            _emit(nc, tc, ctx, x_d, invn_d, w_d, b_d, sel_d, selt_d, selb_d, o_d, T)

    nc.finalize()
    _PROGRAMS[T] = nc
    return nc


def _pack(rows_bf, T):
    """rows [n, C] bf16 -> [128, T*FD] partition-major: partition rb*32+c
    holds row (t*ROWS + rb*FD + j) of channel c at free index t*FD + j."""
    PAD = T * ROWS
    xp = np.zeros((PAD, C), dtype=BF16)
    xp[: rows_bf.shape[0]] = rows_bf
    return np.ascontiguousarray(
        xp.reshape(T, RB, FD, C).transpose(1, 3, 0, 2).reshape(P, T * FD)
    )


def _unpack(slab, n):
    """[128, T*FD] bf16 -> rows [n, C] f32."""
    TF = slab.shape[1] // FD
    return (
        slab.reshape(RB, C, TF, FD)
        .transpose(2, 0, 3, 1)
        .reshape(TF * ROWS, C)[:n]
        .astype(np.float32)
    )


def kernel(feats, seg_ids, weight, bias, num_segments, **_):
    from concourse.bass_utils import run_bass_kernel_spmd

    feats = np.asarray(feats)
    seg = np.asarray(seg_ids)
    w32 = np.asarray(weight, dtype=np.float32).reshape(C)
    b32 = np.asarray(bias, dtype=np.float32).reshape(C)
    S = int(num_segments)
    N = feats.shape[0]

    assert (np.diff(seg) >= 0).all(), "seg_ids must be sorted"
    bounds = np.searchsorted(seg, np.arange(S + 1)).astype(np.int64)
    counts = np.diff(bounds)

    feats_bf = np.asarray(feats, dtype=np.float32).astype(BF16)

    # channel selector: sel[p, c] = (p % 32 == c)
    eye = np.eye(C, dtype=np.float32)
    sel = np.ascontiguousarray(np.tile(eye, (RB, 1)))
    w128 = np.ascontiguousarray(np.tile(w32, RB).reshape(P, 1))
    b128 = np.ascontiguousarray(np.tile(b32, RB).reshape(P, 1))

    out = np.empty((N, C), dtype=np.float32)
    for g0 in range(0, S, NCORES):
        gsegs = list(range(g0, min(g0 + NCORES, S)))
        maxc = max(int(counts[s]) for s in gsegs)
        T = max(1, -(-maxc // ROWS))
        nc = _get_program(T)
        in_maps = []
        for j in range(NCORES):
            n_j = 1
            if j < len(gsegs):
                s = gsegs[j]
                n_j = max(int(counts[s]), 1)
                rows = feats_bf[bounds[s] : bounds[s + 1]]
            else:
                rows = np.zeros((0, C), dtype=BF16)
            in_maps.append(
                {
                    "x": _pack(rows, T),
                    "invn": np.full((P, 1), 1.0 / n_j, dtype=np.float32),
                    "w": w128,
                    "b": b128,
                    "sel": sel,
                    "selt": np.ascontiguousarray(sel.T),
                    "selb": sel.astype(BF16),
                }
            )
        global LAST_RESULTS
        LAST_RESULTS = run_bass_kernel_spmd(nc, in_maps, list(range(NCORES)))
        results = LAST_RESULTS.results
        for j, s in enumerate(gsegs):
            out[bounds[s] : bounds[s + 1]] = _unpack(results[j]["o"], int(counts[s]))
    return out


# revision 46
# speedup vs baseline: 1.0078x; 1.0078x over previous
"""MinkowskiInstanceNorm (segment-reduce instance norm) on 8 Trainium2 cores.

Strategy: seg_ids are sorted, so each segment is a contiguous run of rows.
With num_segments == n_cores == 8, core j owns segment j outright — no
cross-core communication; the host slices rows per segment and stitches
outputs back.

All device traffic is bf16 (the output error budget is ~1e-2 rel; bf16
rounding is ~4e-3 worst-case), which halves HBM bytes vs f32 and lets the
whole input stay resident in SBUF, so pass 2 re-reads nothing:

  - host packs each core's slab CHANNELS-ON-PARTITIONS, partition-major
    bf16 x[128, T*2048] (partition rb*32+c, free index t*2048+j for row
    t*8192 + rb*2048 + j), so any free-dim range is a valid DMA chunk.
  - pass 1 loads in 4-tile (2 MiB) chunks for DMA efficiency, with the
    last few tiles as small chunks so the stats barrier clears early.
    Per chunk: DVE tensor_scalar(copy)+accum_out -> sum(x) (4x bf16);
    sum(x^2) split ~69/31 between ACT Square+accum_out and DVE
    tensor_tensor_reduce(x*x)+accum so both engines track the DMA rate.
  - cross-partition fold (4 row-blocks per channel) is one tiny matmul
    against a replicated 0/1 selector whose output lands per-channel
    values on ALL 128 partitions — no broadcast step; stats chain reads
    the PSUM result directly.
  - pass 2: in-place DVE tensor_scalar (x = x*A[p] - B'[p], 4x bf16) on
    ramped chunks (1,2,4,8.. tiles) stored straight out of the slab,
    alternating the two HW-DGE rings.
"""

from contextlib import ExitStack

import ml_dtypes
import numpy as np

C = 32  # channels
P = 128  # SBUF partitions
RB = P // C  # row blocks per tile (4)
FD = 2048  # rows per partition per tile (free dim)
ROWS = RB * FD  # rows per tile (8192)
NCORES = 8
EPS = 1e-8
BULK = 2  # tiles per bulk load chunk
TAIL = 2  # trailing tiles loaded singly (plus the split last tile)

BF16 = ml_dtypes.bfloat16

_PROGRAMS = {}
LAST_RESULTS = None  # BassKernelResults of the last SPMD launch (for test harness)


def _load_chunks(TW):
    """Pass-1 load chunks as (start, width) over total free width TW.
    Bulk 2-tile chunks taper to single tiles toward the end, and the last
    tile is split so the final stats ops are small and clear right behind
    the last bytes instead of a late chunk serializing engine work."""
    T = -(-TW // FD)
    taper = []  # tile widths, built back-to-front
    rem = T - 1
    for wtiles in [1] * 6:
        if rem <= 0:
            break
        w = min(wtiles, rem)
        taper.append(w)
        rem -= w
    while rem > 0:
        w = min(BULK, rem)
        taper.append(w)
        rem -= w
    chunks = []
    a = 0
    for wtiles in reversed(taper):
        chunks.append((a, wtiles * FD))
        a += wtiles * FD
    # last tile split so its stats clear right behind the final bytes
    tail = min(FD // 8, max(TW - a - 128, 128))
    if TW - a - tail > 0:
        chunks.append((a, TW - a - tail))
        a = TW - tail
    chunks.append((a, TW - a))
    return chunks


def _store_chunks(TW):
    """Pass-2 store chunks: ramp up so the first store issues as soon as
    the affine coefficients land."""
    chunks = []
    a, w = 0, FD
    while a < TW:
        w = min(w, TW - a)
        chunks.append((a, w))
        a += w
        w = min(8 * FD, w * 2)
    return chunks


def _emit(nc, tc, ctx, x_d, w_d, b_d, sel_d, selt_d, selb_d, o_d, TW):
    from concourse import mybir

    dt = mybir.dt
    AX = mybir.AxisListType
    OP = mybir.AluOpType
    AF = mybir.ActivationFunctionType

    xv = x_d.ap()  # [P, T*FD] bf16
    ov = o_d.ap()

    const = ctx.enter_context(tc.tile_pool(name="const", bufs=1))
    scr = ctx.enter_context(tc.tile_pool(name="scr", bufs=1))
    psum = ctx.enter_context(tc.tile_pool(name="psum", bufs=1, space="PSUM"))

    # whole input stays resident across both passes
    slab = const.tile([P, TW], dt.bfloat16)

    # consts ride the scalar (ACT) DGE ring: the sync ring stays a pure
    # back-to-back stream of slab loads from instruction 0
    wt = const.tile([P, 1], dt.float32)
    nc.scalar.dma_start(out=wt[:], in_=w_d.ap())
    bt = const.tile([P, 1], dt.float32)
    nc.scalar.dma_start(out=bt[:], in_=b_d.ap())
    # channel selector sel[p, c] = (p % 32 == c), PRE-SCALED by 1/n on the
    # host, so the fold matmuls produce E[x] / E[x^2] directly: f32 [P,32]
    # folds the f32 sumsq partials; bf16 [P,32] is the stationary operand
    # for the PE's running sum(x/n) (32 columns -> cheap weight load); f32
    # [32,P] (unscaled) broadcasts the final [32,2] affine to all partitions
    sel = const.tile([P, C], dt.float32)
    nc.scalar.dma_start(out=sel[:], in_=sel_d.ap())
    selt = const.tile([C, P], dt.float32)
    nc.scalar.dma_start(out=selt[:], in_=selt_d.ap())
    selb = const.tile([P, C], dt.bfloat16)
    nc.scalar.dma_start(out=selb[:], in_=selb_d.ap())

    loads = _load_chunks(TW)
    nL = len(loads)
    qcola = const.tile([P, nL], dt.float32)  # per-chunk sum(x^2), ACT part
    qcold = const.tile([P, nL], dt.float32)  # per-chunk sum(x^2), DVE part

    dscr = scr.tile([P, BULK * FD], dt.bfloat16)  # DVE squares
    dscr2 = scr.tile([P, BULK * FD], dt.bfloat16)  # DVE scratch (unread)
    ascr = scr.tile([P, BULK * FD], dt.bfloat16)  # ACT scratch (unread)

    epsv = const.tile([P, 1], dt.float32)
    nc.vector.memset(epsv[:], EPS)
    std = const.tile([P, 1], dt.float32)
    # dummy Sqrt up front loads sqrt_and_others (which also contains
    # square) once at t=0, instead of a mid-barrier table switch later
    nc.scalar.activation(std[:], epsv[:], AF.Sqrt, bias=epsv[:])

    # PE accumulates sum(x) (and the channel fold, via the selector) into
    # one PSUM bank: psx[c, f] += sum_p sel[p, c] * x[p, 512k + f].
    # This keeps DVE's per-chunk load under ~80% of the DMA cadence so the
    # chunk-serial latency drains instead of cascading into the barrier.
    psx = psum.tile([C, 512], dt.float32)

    nmm = sum(-(-w // 512) for _, w in loads)
    ka = kd = 0
    mm = 0
    for k, (a, w) in enumerate(loads):
        xt = slab[:, a : a + w]
        nc.sync.dma_start(out=xt, in_=xv[:, a : a + w])
        # sum(x) on the otherwise-idle PE, freeing DVE for sum(x^2)
        for c in range(0, w, 512):
            sw = min(512, w - c)
            nc.tensor.matmul(
                psx[:, :sw],
                lhsT=selb[:],
                rhs=xt[:, c : c + sw],
                start=(mm == 0),
                stop=(mm == nmm - 1),
            )
            mm += 1
        # sum(x^2) split ~48/52 ACT/DVE; shares leave both engines ~35%
        # slack so chunk-serial latency drains instead of cascading
        if w > FD // 2:
            aw = round((0.476 * w) / 128) * 128
        else:
            aw = 0 if kd <= ka else w  # tiny tail chunks: one engine each
        if aw:
            nc.scalar.activation(
                ascr[:, :aw],
                xt[:, :aw],
                AF.Square,
                accum_out=qcola[:, ka : ka + 1],
            )
            ka += 1
        if aw < w:
            # DVE: square into scratch, then sum it with the accumulator
            # (tensor_tensor_reduce is broken in this runtime)
            dw = w - aw
            nc.vector.tensor_mul(dscr[:, :dw], xt[:, aw:], xt[:, aw:])
            nc.vector.tensor_scalar(
                out=dscr2[:, :dw],
                in0=dscr[:, :dw],
                scalar1=1.0,
                scalar2=0.0,
                op0=OP.mult,
                op1=OP.add,
                accum_out=qcold[:, kd : kd + 1],
            )
            kd += 1

    # mean = E[x]: ACT folds the PE accumulator (copy + free-dim
    # accumulator, reads PSUM directly; 1/n is already in the selector)
    # while DVE reduces the sumsq partials in parallel
    mean = const.tile([C, 1], dt.float32)
    pscr = scr.tile([C, 512], dt.float32)  # unread
    nc.scalar.activation(pscr[:], psx[:], AF.Copy, accum_out=mean[:])

    # per-chunk sumsq partials -> [P, 1]
    qa = const.tile([P, 2], dt.float32)
    nc.vector.tensor_reduce(out=qa[:, 0:1], in_=qcola[:, :ka], axis=AX.X, op=OP.add)
    nc.vector.tensor_reduce(out=qa[:, 1:2], in_=qcold[:, :kd], axis=AX.X, op=OP.add)
    q = const.tile([P, 1], dt.float32)
    nc.vector.tensor_add(q[:], qa[:, 0:1], qa[:, 1:2])

    # fold the RB row-blocks -> E[x^2] [32, 1] (1/n via scaled selector)
    tot = psum.tile([C, 1], dt.float32)
    nc.tensor.matmul(tot[:], lhsT=sel[:], rhs=q[:], start=True, stop=True)

    # nvar = mean^2 - E[x^2] = -var, then std = sqrt(-nvar + eps)
    nvar = const.tile([C, 1], dt.float32)
    nc.vector.scalar_tensor_tensor(
        out=nvar[:], in0=mean[:], scalar=mean[:], in1=tot[:],
        op0=OP.mult, op1=OP.subtract,
    )
    nc.scalar.activation(
        std[:C, :], nvar[:], AF.Sqrt, bias=epsv[:C, :], scale=-1.0
    )
    istd = const.tile([C, 1], dt.float32)
    nc.vector.reciprocal(istd[:], std[:C, :])
    # ab32 = [A | B']: A = w/std, B' = mean*A - b; out = x*A - B'
    ab32 = const.tile([C, 2], dt.float32)
    nc.vector.tensor_scalar(
        out=ab32[:, 0:1], in0=wt[:C, :], scalar1=istd[:], scalar2=None,
        op0=OP.mult,
    )
    nc.vector.scalar_tensor_tensor(
        out=ab32[:, 1:2], in0=mean[:], scalar=ab32[:, 0:1], in1=bt[:C, :],
        op0=OP.mult, op1=OP.subtract,
    )
    # broadcast [32, 2] -> all 128 partitions (transposed-selector matmul)
    abps = psum.tile([P, 2], dt.float32)
    nc.tensor.matmul(abps[:], lhsT=selt[:], rhs=ab32[:], start=True, stop=True)
    # DVE copy: the normalize runs on DVE too, so no cross-engine sync
    ab = const.tile([P, 2], dt.float32)
    nc.vector.tensor_copy(ab[:], abps[:])

    for k, (a, w) in enumerate(_store_chunks(TW)):
        xt = slab[:, a : a + w]
        # in-place normalize: x = x*A - B' (4x bf16, single-src)
        nc.vector.tensor_scalar(
            out=xt,
            in0=xt,
            scalar1=ab[:, 0:1],
            scalar2=ab[:, 1:2],
            op0=OP.mult,
            op1=OP.subtract,
        )
        # alternate the two HW-DGE rings (sync is idle during pass 2) so
        # per-store issue cost never gates the store stream
        eng = nc.scalar if k % 2 == 0 else nc.sync
        eng.dma_start(out=ov[:, a : a + w], in_=xt)


def _get_program(TW):
    if TW in _PROGRAMS:
        return _PROGRAMS[TW]
    import concourse.tile as tile
    from concourse import bacc, mybir

    dt = mybir.dt
    nc = bacc.Bacc(
        "TRN2",
        target_bir_lowering=False,
        debug=False,
        enable_asserts=False,
        num_devices=NCORES,
    )
    x_d = nc.dram_tensor("x", [P, TW], dt.bfloat16, kind="ExternalInput")
    w_d = nc.dram_tensor("w", [P, 1], dt.float32, kind="ExternalInput")
    b_d = nc.dram_tensor("b", [P, 1], dt.float32, kind="ExternalInput")
    sel_d = nc.dram_tensor("sel", [P, C], dt.float32, kind="ExternalInput")
    selt_d = nc.dram_tensor("selt", [C, P], dt.float32, kind="ExternalInput")
    selb_d = nc.dram_tensor("selb", [P, C], dt.bfloat16, kind="ExternalInput")
    o_d = nc.dram_tensor("o", [P, TW], dt.bfloat16, kind="ExternalOutput")

    with tile.TileContext(nc) as tc:
        with ExitStack() as ctx:
            _emit(nc, tc, ctx, x_d, w_d, b_d, sel_d, selt_d, selb_d, o_d, TW)

    nc.finalize()
    _PROGRAMS[TW] = nc
    return nc


def _pack(rows_bf, T, FDL):
    """rows [n, C] bf16 -> [128, (T-1)*FD + FDL] partition-major: partition
    rb*32+c holds row (t*ROWS + rb*w_t + j) of channel c at free index
    t*FD + j, where w_t = FD for full tiles and FDL for the last tile."""
    nmain = (T - 1) * ROWS
    xp = np.zeros((nmain + RB * FDL, C), dtype=BF16)
    xp[: rows_bf.shape[0]] = rows_bf
    main = (
        xp[:nmain].reshape(T - 1, RB, FD, C).transpose(1, 3, 0, 2)
        .reshape(P, (T - 1) * FD)
    )
    last = (
        xp[nmain:].reshape(1, RB, FDL, C).transpose(1, 3, 0, 2)
        .reshape(P, FDL)
    )
    return np.ascontiguousarray(np.concatenate([main, last], axis=1))


def _unpack(slab, n, T, FDL):
    """[128, (T-1)*FD + FDL] bf16 -> rows [n, C] f32."""
    main = (
        slab[:, : (T - 1) * FD]
        .reshape(RB, C, T - 1, FD)
        .transpose(2, 0, 3, 1)
        .reshape((T - 1) * ROWS, C)
    )
    last = (
        slab[:, (T - 1) * FD :]
        .reshape(RB, C, 1, FDL)
        .transpose(2, 0, 3, 1)
        .reshape(RB * FDL, C)
    )
    return np.concatenate([main, last], axis=0)[:n].astype(np.float32)


def kernel(feats, seg_ids, weight, bias, num_segments, **_):
    from concourse.bass_utils import run_bass_kernel_spmd

    feats = np.asarray(feats)
    seg = np.asarray(seg_ids)
    w32 = np.asarray(weight, dtype=np.float32).reshape(C)
    b32 = np.asarray(bias, dtype=np.float32).reshape(C)
    S = int(num_segments)
    N = feats.shape[0]

    assert (np.diff(seg) >= 0).all(), "seg_ids must be sorted"
    bounds = np.searchsorted(seg, np.arange(S + 1)).astype(np.int64)
    counts = np.diff(bounds)

    feats_bf = np.asarray(feats, dtype=np.float32).astype(BF16)

    # channel selector: sel[p, c] = (p % 32 == c)
    eye = np.eye(C, dtype=np.float32)
    sel = np.ascontiguousarray(np.tile(eye, (RB, 1)))
    w128 = np.ascontiguousarray(np.tile(w32, RB).reshape(P, 1))
    b128 = np.ascontiguousarray(np.tile(b32, RB).reshape(P, 1))

    out = np.empty((N, C), dtype=np.float32)
    for g0 in range(0, S, NCORES):
        gsegs = list(range(g0, min(g0 + NCORES, S)))
        maxc = max(int(counts[s]) for s in gsegs)
        T = max(1, -(-maxc // ROWS))
        # last tile trimmed to the actual row count (128-col granularity)
        FDL = min(FD, max(128, -(-(maxc - (T - 1) * ROWS) // (RB * 128)) * 128))
        TW = (T - 1) * FD + FDL
        nc = _get_program(TW)
        in_maps = []
        for j in range(NCORES):
            n_j = 1
            if j < len(gsegs):
                s = gsegs[j]
                n_j = max(int(counts[s]), 1)
                rows = feats_bf[bounds[s] : bounds[s + 1]]
            else:
                rows = np.zeros((0, C), dtype=BF16)
            seln = (sel / n_j).astype(np.float32)
            in_maps.append(
                {
                    "x": _pack(rows, T, FDL),
                    "w": w128,
                    "b": b128,
                    "sel": seln,
                    "selt": np.ascontiguousarray(sel.T),
                    "selb": seln.astype(BF16),
                }
            )
        global LAST_RESULTS
        LAST_RESULTS = run_bass_kernel_spmd(nc, in_maps, list(range(NCORES)))
        results = LAST_RESULTS.results
        for j, s in enumerate(gsegs):
            out[bounds[s] : bounds[s + 1]] = _unpack(
                results[j]["o"], int(counts[s]), T, FDL
            )
    return out


# revision 51
# speedup vs baseline: 1.0097x; 1.0019x over previous
"""MinkowskiInstanceNorm (segment-reduce instance norm) on 8 Trainium2 cores.

Strategy: seg_ids are sorted, so each segment is a contiguous run of rows.
With num_segments == n_cores == 8, core j owns segment j outright — no
cross-core communication; the host slices rows per segment and stitches
outputs back.

All device traffic is bf16 (the output error budget is ~1e-2 rel; bf16
rounding is ~4e-3 worst-case), which halves HBM bytes vs f32 and lets the
whole input stay resident in SBUF, so pass 2 re-reads nothing:

  - host packs each core's slab CHANNELS-ON-PARTITIONS, partition-major
    bf16 x[128, T*2048] (partition rb*32+c, free index t*2048+j for row
    t*8192 + rb*2048 + j), so any free-dim range is a valid DMA chunk.
  - pass 1 loads in 2-tile (1 MiB) chunks tapering to single tiles (the
    last tile split again) so the final stats ops are small and clear
    right behind the last bytes. Per chunk, three engines split the
    stats: the otherwise-idle PE accumulates sum(x) into PSUM via a
    0/1-selector matmul (which also folds the 4 row-blocks per channel
    and applies 1/n, pre-scaled into the selector by the host); sum(x^2)
    is split ~48/52 between ACT Square+accum_out and DVE square +
    tensor_scalar+accum_out. Every engine stays ~35% under the DMA
    cadence, so chunk-serial latency drains instead of piling into the
    stats barrier.
  - the stats chain runs on 32 partitions (ACT folds the PE accumulator
    with a copy+accumulate straight out of PSUM) and one tiny transposed-
    selector matmul broadcasts the final [32, 2] affine to all 128.
  - pass 2: in-place DVE tensor_scalar (x = x*A[p] - B'[p], 4x bf16) on
    ramped chunks (0.25, 1, 4, 8, 8.. tiles) stored straight out of the
    slab, alternating the two HW-DGE rings.
  - the last tile's free width is trimmed to the actual row count (FDL),
    saving ~1.2% of bytes on this shape.
"""

from contextlib import ExitStack

import ml_dtypes
import numpy as np

C = 32  # channels
P = 128  # SBUF partitions
RB = P // C  # row blocks per tile (4)
FD = 2048  # rows per partition per tile (free dim)
ROWS = RB * FD  # rows per tile (8192)
NCORES = 8
EPS = 1e-8
BULK = 2  # tiles per bulk load chunk
TAIL = 2  # trailing tiles loaded singly (plus the split last tile)

BF16 = ml_dtypes.bfloat16

_PROGRAMS = {}
LAST_RESULTS = None  # BassKernelResults of the last SPMD launch (for test harness)


def _load_chunks(TW):
    """Pass-1 load chunks as (start, width) over total free width TW.
    Bulk 2-tile chunks taper to single tiles toward the end, and the last
    tile is split so the final stats ops are small and clear right behind
    the last bytes instead of a late chunk serializing engine work."""
    T = -(-TW // FD)
    taper = []  # tile widths, built back-to-front
    rem = T - 1
    for wtiles in [1] * 6:
        if rem <= 0:
            break
        w = min(wtiles, rem)
        taper.append(w)
        rem -= w
    while rem > 0:
        w = min(BULK, rem)
        taper.append(w)
        rem -= w
    chunks = []
    a = 0
    for wtiles in reversed(taper):
        chunks.append((a, wtiles * FD))
        a += wtiles * FD
    # last tile split so its stats clear right behind the final bytes
    tail = min(FD // 8, max(TW - a - 128, 128))
    if TW - a - tail > 0:
        chunks.append((a, TW - a - tail))
        a = TW - tail
    chunks.append((a, TW - a))
    return chunks


def _store_chunks(TW):
    """Pass-2 store chunks: ramp up so the first store issues as soon as
    the affine coefficients land."""
    chunks = []
    a, w = 0, FD // 4
    while a < TW:
        w = min(w, TW - a)
        chunks.append((a, w))
        a += w
        w = min(8 * FD, w * 2)
    return chunks


def _emit(nc, tc, ctx, x_d, w_d, b_d, sel_d, selt_d, selb_d, o_d, TW):
    from concourse import mybir

    dt = mybir.dt
    AX = mybir.AxisListType
    OP = mybir.AluOpType
    AF = mybir.ActivationFunctionType

    xv = x_d.ap()  # [P, T*FD] bf16
    ov = o_d.ap()

    const = ctx.enter_context(tc.tile_pool(name="const", bufs=1))
    scr = ctx.enter_context(tc.tile_pool(name="scr", bufs=1))
    psum = ctx.enter_context(tc.tile_pool(name="psum", bufs=1, space="PSUM"))

    # whole input stays resident across both passes
    slab = const.tile([P, TW], dt.bfloat16)

    # consts ride the scalar (ACT) DGE ring: the sync ring stays a pure
    # back-to-back stream of slab loads from instruction 0
    wt = const.tile([P, 1], dt.float32)
    nc.scalar.dma_start(out=wt[:], in_=w_d.ap())
    bt = const.tile([P, 1], dt.float32)
    nc.scalar.dma_start(out=bt[:], in_=b_d.ap())
    # channel selector sel[p, c] = (p % 32 == c), PRE-SCALED by 1/n on the
    # host, so the fold matmuls produce E[x] / E[x^2] directly: f32 [P,32]
    # folds the f32 sumsq partials; bf16 [P,32] is the stationary operand
    # for the PE's running sum(x/n) (32 columns -> cheap weight load); f32
    # [32,P] (unscaled) broadcasts the final [32,2] affine to all partitions
    sel = const.tile([P, C], dt.float32)
    nc.scalar.dma_start(out=sel[:], in_=sel_d.ap())
    selt = const.tile([C, P], dt.float32)
    nc.scalar.dma_start(out=selt[:], in_=selt_d.ap())
    selb = const.tile([P, C], dt.bfloat16)
    nc.scalar.dma_start(out=selb[:], in_=selb_d.ap())

    loads = _load_chunks(TW)
    nL = len(loads)
    qcola = const.tile([P, nL], dt.float32)  # per-chunk sum(x^2), ACT part
    qcold = const.tile([P, nL], dt.float32)  # per-chunk sum(x^2), DVE part

    dscr = scr.tile([P, BULK * FD], dt.bfloat16)  # DVE squares
    dscr2 = scr.tile([P, BULK * FD], dt.bfloat16)  # DVE scratch (unread)
    ascr = scr.tile([P, BULK * FD], dt.bfloat16)  # ACT scratch (unread)

    epsv = const.tile([P, 1], dt.float32)
    nc.vector.memset(epsv[:], EPS)
    std = const.tile([P, 1], dt.float32)
    # dummy Sqrt up front loads sqrt_and_others (which also contains
    # square) once at t=0, instead of a mid-barrier table switch later
    nc.scalar.activation(std[:], epsv[:], AF.Sqrt, bias=epsv[:])

    # PE accumulates sum(x) (and the channel fold, via the selector) into
    # one PSUM bank: psx[c, f] += sum_p sel[p, c] * x[p, 512k + f].
    # This keeps DVE's per-chunk load under ~80% of the DMA cadence so the
    # chunk-serial latency drains instead of cascading into the barrier.
    psx = psum.tile([C, 512], dt.float32)

    nmm = sum(-(-w // 512) for _, w in loads)
    ka = kd = 0
    mm = 0
    for k, (a, w) in enumerate(loads):
        xt = slab[:, a : a + w]
        nc.sync.dma_start(out=xt, in_=xv[:, a : a + w])
        # sum(x) on the otherwise-idle PE, freeing DVE for sum(x^2)
        for c in range(0, w, 512):
            sw = min(512, w - c)
            nc.tensor.matmul(
                psx[:, :sw],
                lhsT=selb[:],
                rhs=xt[:, c : c + sw],
                start=(mm == 0),
                stop=(mm == nmm - 1),
            )
            mm += 1
        # sum(x^2) split ~48/52 ACT/DVE; shares leave both engines ~35%
        # slack so chunk-serial latency drains instead of cascading
        if w > FD // 2:
            aw = round((0.476 * w) / 128) * 128
        else:
            aw = 0 if kd <= ka else w  # tiny tail chunks: one engine each
        if aw:
            nc.scalar.activation(
                ascr[:, :aw],
                xt[:, :aw],
                AF.Square,
                accum_out=qcola[:, ka : ka + 1],
            )
            ka += 1
        if aw < w:
            # DVE: square into scratch, then sum it with the accumulator
            # (tensor_tensor_reduce is broken in this runtime)
            dw = w - aw
            nc.vector.tensor_mul(dscr[:, :dw], xt[:, aw:], xt[:, aw:])
            nc.vector.tensor_scalar(
                out=dscr2[:, :dw],
                in0=dscr[:, :dw],
                scalar1=1.0,
                scalar2=0.0,
                op0=OP.mult,
                op1=OP.add,
                accum_out=qcold[:, kd : kd + 1],
            )
            kd += 1

    # mean = E[x]: ACT folds the PE accumulator (copy + free-dim
    # accumulator, reads PSUM directly; 1/n is already in the selector)
    # while DVE reduces the sumsq partials in parallel
    mean = const.tile([C, 1], dt.float32)
    pscr = scr.tile([C, 512], dt.float32)  # unread
    nc.scalar.activation(pscr[:], psx[:], AF.Copy, accum_out=mean[:])

    # per-chunk sumsq partials -> [P, 1]
    qa = const.tile([P, 2], dt.float32)
    nc.vector.tensor_reduce(out=qa[:, 0:1], in_=qcola[:, :ka], axis=AX.X, op=OP.add)
    nc.vector.tensor_reduce(out=qa[:, 1:2], in_=qcold[:, :kd], axis=AX.X, op=OP.add)
    q = const.tile([P, 1], dt.float32)
    nc.vector.tensor_add(q[:], qa[:, 0:1], qa[:, 1:2])

    # fold the RB row-blocks -> E[x^2] [32, 1] (1/n via scaled selector)
    tot = psum.tile([C, 1], dt.float32)
    nc.tensor.matmul(tot[:], lhsT=sel[:], rhs=q[:], start=True, stop=True)

    # nvar = mean^2 - E[x^2] = -var, then std = sqrt(-nvar + eps)
    nvar = const.tile([C, 1], dt.float32)
    nc.vector.scalar_tensor_tensor(
        out=nvar[:], in0=mean[:], scalar=mean[:], in1=tot[:],
        op0=OP.mult, op1=OP.subtract,
    )
    nc.scalar.activation(
        std[:C, :], nvar[:], AF.Sqrt, bias=epsv[:C, :], scale=-1.0
    )
    istd = const.tile([C, 1], dt.float32)
    nc.vector.reciprocal(istd[:], std[:C, :])
    # ab32 = [A | B']: A = w/std, B' = mean*A - b; out = x*A - B'
    ab32 = const.tile([C, 2], dt.float32)
    nc.vector.tensor_scalar(
        out=ab32[:, 0:1], in0=wt[:C, :], scalar1=istd[:], scalar2=None,
        op0=OP.mult,
    )
    nc.vector.scalar_tensor_tensor(
        out=ab32[:, 1:2], in0=mean[:], scalar=ab32[:, 0:1], in1=bt[:C, :],
        op0=OP.mult, op1=OP.subtract,
    )
    # broadcast [32, 2] -> all 128 partitions (transposed-selector matmul)
    abps = psum.tile([P, 2], dt.float32)
    nc.tensor.matmul(abps[:], lhsT=selt[:], rhs=ab32[:], start=True, stop=True)
    # DVE copy: the normalize runs on DVE too, so no cross-engine sync
    ab = const.tile([P, 2], dt.float32)
    nc.vector.tensor_copy(ab[:], abps[:])

    for k, (a, w) in enumerate(_store_chunks(TW)):
        xt = slab[:, a : a + w]
        # in-place normalize: x = x*A - B' (4x bf16, single-src)
        nc.vector.tensor_scalar(
            out=xt,
            in0=xt,
            scalar1=ab[:, 0:1],
            scalar2=ab[:, 1:2],
            op0=OP.mult,
            op1=OP.subtract,
        )
        # alternate the two HW-DGE rings (sync is idle during pass 2) so
        # per-store issue cost never gates the store stream
        eng = nc.scalar if k % 2 == 0 else nc.sync
        eng.dma_start(out=ov[:, a : a + w], in_=xt)


def _get_program(TW):
    if TW in _PROGRAMS:
        return _PROGRAMS[TW]
    import concourse.tile as tile
    from concourse import bacc, mybir

    dt = mybir.dt
    nc = bacc.Bacc(
        "TRN2",
        target_bir_lowering=False,
        debug=False,
        enable_asserts=False,
        num_devices=NCORES,
    )
    x_d = nc.dram_tensor("x", [P, TW], dt.bfloat16, kind="ExternalInput")
    w_d = nc.dram_tensor("w", [P, 1], dt.float32, kind="ExternalInput")
    b_d = nc.dram_tensor("b", [P, 1], dt.float32, kind="ExternalInput")
    sel_d = nc.dram_tensor("sel", [P, C], dt.float32, kind="ExternalInput")
    selt_d = nc.dram_tensor("selt", [C, P], dt.float32, kind="ExternalInput")
    selb_d = nc.dram_tensor("selb", [P, C], dt.bfloat16, kind="ExternalInput")
    o_d = nc.dram_tensor("o", [P, TW], dt.bfloat16, kind="ExternalOutput")

    with tile.TileContext(nc) as tc:
        with ExitStack() as ctx:
            _emit(nc, tc, ctx, x_d, w_d, b_d, sel_d, selt_d, selb_d, o_d, TW)

    nc.finalize()
    _PROGRAMS[TW] = nc
    return nc


def _pack(rows_bf, T, FDL):
    """rows [n, C] bf16 -> [128, (T-1)*FD + FDL] partition-major: partition
    rb*32+c holds row (t*ROWS + rb*w_t + j) of channel c at free index
    t*FD + j, where w_t = FD for full tiles and FDL for the last tile."""
    nmain = (T - 1) * ROWS
    xp = np.zeros((nmain + RB * FDL, C), dtype=BF16)
    xp[: rows_bf.shape[0]] = rows_bf
    main = (
        xp[:nmain].reshape(T - 1, RB, FD, C).transpose(1, 3, 0, 2)
        .reshape(P, (T - 1) * FD)
    )
    last = (
        xp[nmain:].reshape(1, RB, FDL, C).transpose(1, 3, 0, 2)
        .reshape(P, FDL)
    )
    return np.ascontiguousarray(np.concatenate([main, last], axis=1))


def _unpack(slab, n, T, FDL):
    """[128, (T-1)*FD + FDL] bf16 -> rows [n, C] f32."""
    main = (
        slab[:, : (T - 1) * FD]
        .reshape(RB, C, T - 1, FD)
        .transpose(2, 0, 3, 1)
        .reshape((T - 1) * ROWS, C)
    )
    last = (
        slab[:, (T - 1) * FD :]
        .reshape(RB, C, 1, FDL)
        .transpose(2, 0, 3, 1)
        .reshape(RB * FDL, C)
    )
    return np.concatenate([main, last], axis=0)[:n].astype(np.float32)


def kernel(feats, seg_ids, weight, bias, num_segments, **_):
    from concourse.bass_utils import run_bass_kernel_spmd

    feats = np.asarray(feats)
    seg = np.asarray(seg_ids)
    w32 = np.asarray(weight, dtype=np.float32).reshape(C)
    b32 = np.asarray(bias, dtype=np.float32).reshape(C)
    S = int(num_segments)
    N = feats.shape[0]

    assert (np.diff(seg) >= 0).all(), "seg_ids must be sorted"
    bounds = np.searchsorted(seg, np.arange(S + 1)).astype(np.int64)
    counts = np.diff(bounds)

    feats_bf = np.asarray(feats, dtype=np.float32).astype(BF16)

    # channel selector: sel[p, c] = (p % 32 == c)
    eye = np.eye(C, dtype=np.float32)
    sel = np.ascontiguousarray(np.tile(eye, (RB, 1)))
    w128 = np.ascontiguousarray(np.tile(w32, RB).reshape(P, 1))
    b128 = np.ascontiguousarray(np.tile(b32, RB).reshape(P, 1))

    out = np.empty((N, C), dtype=np.float32)
    for g0 in range(0, S, NCORES):
        gsegs = list(range(g0, min(g0 + NCORES, S)))
        maxc = max(int(counts[s]) for s in gsegs)
        T = max(1, -(-maxc // ROWS))
        # last tile trimmed to the actual row count (128-col granularity)
        FDL = min(FD, max(128, -(-(maxc - (T - 1) * ROWS) // (RB * 128)) * 128))
        TW = (T - 1) * FD + FDL
        nc = _get_program(TW)
        in_maps = []
        for j in range(NCORES):
            n_j = 1
            if j < len(gsegs):
                s = gsegs[j]
                n_j = max(int(counts[s]), 1)
                rows = feats_bf[bounds[s] : bounds[s + 1]]
            else:
                rows = np.zeros((0, C), dtype=BF16)
            seln = (sel / n_j).astype(np.float32)
            in_maps.append(
                {
                    "x": _pack(rows, T, FDL),
                    "w": w128,
                    "b": b128,
                    "sel": seln,
                    "selt": np.ascontiguousarray(sel.T),
                    "selb": seln.astype(BF16),
                }
            )
        global LAST_RESULTS
        LAST_RESULTS = run_bass_kernel_spmd(nc, in_maps, list(range(NCORES)))
        results = LAST_RESULTS.results
        for j, s in enumerate(gsegs):
            out[bounds[s] : bounds[s + 1]] = _unpack(
                results[j]["o"], int(counts[s]), T, FDL
            )
    return out
